# revision 2
# baseline (speedup 1.0000x reference)
"""GRU decoder Trainium2 kernel (data-parallel over batch, 8 cores).

Reference computation (per step t, PyTorch nn.GRU gate order r,z,n):
    gi = x @ w_ih.T + b_ih ; gh = h @ w_hh.T + b_hh
    r = sig(i_r + h_r); z = sig(i_z + h_z); n = tanh(i_n + r * h_n)
    h' = (1-z)*n + z*h ; y = h' @ w_fc.T + b_fc ; x <- y
Shapes: H=1024, O=768, B=256, T=256.  Each core handles 32 batch rows.

Device strategy (per core, per step):
  * batch (32) lives in the PE stationary-operand free dim (M=32);
    4 column-groups of the 128x128 array run concurrently via
    tile_position=(0,32j), each j handling a different slice of the
    gate/output feature dim.  Weights stream as the moving operand.
  * packed layouts: a [128, C] tile holds rows 32j+b = (feature-chunk j,
    batch b).  All 128 vector/scalar lanes stay busy for elementwise work.
  * feedback transposes (h', y -> stationary operands of the next step)
    are done with PE transpose + a psum->sbuf cast copy.
  * b_fc is folded into the gate biases (x enters the gates as y-b_fc);
    the host adds b_fc back to the final output.
"""

import numpy as np
import ml_dtypes

import concourse.bass as bass
import concourse.bacc as bacc
import concourse.tile as tile
from concourse import mybir
from concourse.bass_utils import run_bass_kernel_spmd

H = 1024
O = 768
B = 256
T = 256
NCORES = 8
BC = B // NCORES  # 32 batch rows per core

KX = O // 128  # 6 x-contraction chunks
KH = H // 128  # 8 h-contraction chunks
KG = KX + KH   # 14 gate-contraction chunks

F32 = mybir.dt.float32
BF16 = mybir.dt.bfloat16
AF = mybir.ActivationFunctionType
ALU = mybir.AluOpType

_COMPILED = None


NBF = KG * 3072 + KH * O + 128 + 64  # WG | WF | X0I | X1I
NF32 = 512 + 256 + 256 + 128 + 256   # BRZ | BIN | BHN | IDT | HPI


def _build_nc():
    nc = bacc.Bacc("TRN2", target_bir_lowering=False, debug=False, num_devices=NCORES)

    cbf = nc.declare_dram_parameter("CBF", [128, NBF], BF16, isOutput=False)
    cf32 = nc.declare_dram_parameter("CF32", [128, NF32], F32, isOutput=False)
    o0 = nc.declare_dram_parameter("O0", [T, 128, 128], F32, isOutput=True)
    o1 = nc.declare_dram_parameter("O1", [T, 64, 128], F32, isOutput=True)

    with tile.TileContext(nc) as tc:
        with (
            tc.tile_pool(name="wpool", bufs=1) as wpool,
            tc.tile_pool(name="state", bufs=2) as spool,
            tc.tile_pool(name="act", bufs=2) as apool,
            tc.tile_pool(name="psA", bufs=2, space="PSUM") as psA,
            tc.tile_pool(name="psB", bufs=1, space="PSUM") as psB,
        ):
            # ---- constants into SBUF (2 DMAs to cap sync-wait fan-in) ----
            CB = wpool.tile([128, NBF], BF16, tag="CB")
            CF = wpool.tile([128, NF32], F32, tag="CF")
            nc.sync.dma_start(CB[:], cbf[:])
            nc.sync.dma_start(CF[:], cf32[:])
            WG = CB[:, 0 : KG * 3072]
            WF = CB[:, KG * 3072 : KG * 3072 + KH * O]
            X0c = CB[:, KG * 3072 + KH * O : KG * 3072 + KH * O + 128]
            X1c = CB[:, KG * 3072 + KH * O + 128 : NBF]
            BRZ = CF[:, 0:512]
            BIN = CF[:, 512:768]
            BHN = CF[:, 768:1024]
            IDT = CF[:, 1024:1152]
            HPc = CF[:, 1152:1408]

            # ---- state tiles (rotate via tags) ----
            X0 = spool.tile([128, 128], BF16, tag="X0")
            X1 = spool.tile([128, 64], BF16, tag="X1")
            Hp = spool.tile([128, 256], F32, tag="Hp")
            nc.vector.tensor_copy(X0[:], X0c)
            nc.vector.tensor_copy(X1[:], X1c)
            nc.vector.tensor_copy(Hp[:], HPc)

            def transpose_h(hp_tile):
                """Hp [128,256] -> hsb0/hsb1 [128,128] bf16 (h chunks)."""
                tp = psB.tile([128, 256], F32, tag="tp")
                nc.tensor.transpose(tp[:, 0:128], hp_tile[:, 0:128], IDT)
                nc.tensor.transpose(tp[:, 128:256], hp_tile[:, 128:256], IDT)
                h0 = spool.tile([128, 128], BF16, tag="hsb0")
                h1 = spool.tile([128, 128], BF16, tag="hsb1")
                nc.scalar.activation(h0[:], tp[:, 0:128], AF.Copy)
                nc.scalar.activation(h1[:], tp[:, 128:256], AF.Copy)
                return h0, h1

            hsb0, hsb1 = transpose_h(Hp)

            for t in range(T):
                # ---------- gate matmuls ----------
                grz = psA.tile([128, 512], F32, tag="grz")
                gin = psB.tile([128, 256], F32, tag="gin")
                ghn = psB.tile([128, 256], F32, tag="ghn")
                for k in range(KG):
                    if k < 4:
                        lhsT = X0[:, 32 * k : 32 * k + 32]
                    elif k < 6:
                        lhsT = X1[:, 32 * (k - 4) : 32 * (k - 4) + 32]
                    else:
                        j = (k - 6) // 2
                        src = hsb0 if (k - 6) % 2 == 0 else hsb1
                        lhsT = src[:, 32 * j : 32 * j + 32]
                    wofs = k * 3072
                    for j in range(4):
                        nc.tensor.matmul(
                            grz[32 * j : 32 * j + 32, :],
                            lhsT,
                            WG[:, wofs + 768 * j : wofs + 768 * j + 512],
                            start=(k == 0),
                            stop=(k == KG - 1),
                            tile_position=(0, 32 * j),
                        )
                        nrhs = WG[:, wofs + 768 * j + 512 : wofs + 768 * (j + 1)]
                        if k < KX:
                            nc.tensor.matmul(
                                gin[32 * j : 32 * j + 32, :],
                                lhsT,
                                nrhs,
                                start=(k == 0),
                                stop=(k == KX - 1),
                                tile_position=(0, 32 * j),
                            )
                        else:
                            nc.tensor.matmul(
                                ghn[32 * j : 32 * j + 32, :],
                                lhsT,
                                nrhs,
                                start=(k == KX),
                                stop=(k == KG - 1),
                                tile_position=(0, 32 * j),
                            )

                # ---------- gate activations ----------
                trz = apool.tile([128, 512], F32, tag="trz")
                nc.vector.tensor_tensor(trz[:], grz[:], BRZ[:], ALU.add)
                rz = apool.tile([128, 512], F32, tag="rz")
                nc.scalar.activation(rz[:], trz[:], AF.Sigmoid)

                hnb = apool.tile([128, 256], F32, tag="hnb")
                nc.vector.tensor_tensor(hnb[:], ghn[:], BHN[:], ALU.add)
                rt = apool.tile([128, 256], F32, tag="rt")
                nc.vector.tensor_tensor(rt[:], rz[:, 0:256], hnb[:], ALU.mult)
                ni = apool.tile([128, 256], F32, tag="ni")
                nc.vector.tensor_tensor(ni[:], gin[:], BIN[:], ALU.add)
                ns = apool.tile([128, 256], F32, tag="ns")
                nc.vector.tensor_tensor(ns[:], ni[:], rt[:], ALU.add)
                n = apool.tile([128, 256], F32, tag="n")
                nc.scalar.activation(n[:], ns[:], AF.Tanh)

                # h' = n + z*(h - n)
                d = apool.tile([128, 256], F32, tag="d")
                nc.vector.tensor_tensor(d[:], Hp[:], n[:], ALU.subtract)
                e = apool.tile([128, 256], F32, tag="e")
                nc.vector.tensor_tensor(e[:], rz[:, 256:512], d[:], ALU.mult)
                Hp = spool.tile([128, 256], F32, tag="Hp")
                nc.vector.tensor_tensor(Hp[:], n[:], e[:], ALU.add)

                # ---------- h' transpose (for y matmul + next-step gates) ----
                hsb0, hsb1 = transpose_h(Hp)

                # ---------- y matmul ----------
                yp0 = psB.tile([128, 128], F32, tag="yp0")
                yp1 = psB.tile([64, 128], F32, tag="yp1")
                for k in range(KH):
                    j = k // 2
                    src = hsb0 if k % 2 == 0 else hsb1
                    lhsT = src[:, 32 * j : 32 * j + 32]
                    wofs = k * O
                    for j2 in range(4):
                        nc.tensor.matmul(
                            yp0[32 * j2 : 32 * j2 + 32, :],
                            lhsT,
                            WF[:, wofs + 128 * j2 : wofs + 128 * j2 + 128],
                            start=(k == 0),
                            stop=(k == KH - 1),
                            tile_position=(0, 32 * j2),
                        )
                    for j2 in range(2):
                        nc.tensor.matmul(
                            yp1[32 * j2 : 32 * j2 + 32, :],
                            lhsT,
                            WF[:, wofs + 512 + 128 * j2 : wofs + 512 + 128 * j2 + 128],
                            start=(k == 0),
                            stop=(k == KH - 1),
                            tile_position=(0, 32 * j2),
                        )

                # ---------- y out + feedback transpose ----------
                ys0 = apool.tile([128, 128], F32, tag="ys0")
                ys1 = apool.tile([64, 128], F32, tag="ys1")
                nc.scalar.activation(ys0[:], yp0[:], AF.Copy)
                nc.scalar.activation(ys1[:], yp1[:], AF.Copy)
                nc.sync.dma_start(o0[t], ys0[:])
                nc.sync.dma_start(o1[t], ys1[:])

                xp = psB.tile([128, 192], F32, tag="xp")
                nc.tensor.transpose(xp[:, 0:128], ys0[:], IDT)
                nc.tensor.transpose(xp[:, 128:192], ys1[:], IDT[0:64, 0:64])
                X0 = spool.tile([128, 128], BF16, tag="X0")
                X1 = spool.tile([128, 64], BF16, tag="X1")
                nc.vector.tensor_copy(X0[:], xp[:, 0:128])
                nc.vector.tensor_copy(X1[:], xp[:, 128:192])

    nc.compile()
    return nc


def _prep_shared(w_ih, w_hh, b_ih, b_hh, w_fc, b_fc):
    """Host-side weight/bias rearrangement shared by all cores."""
    wcat = np.concatenate([w_ih.T, w_hh.T], axis=0)  # [1792, 3072] f32
    blocks = []
    for k in range(KG):
        rows = wcat[128 * k : 128 * k + 128]  # [128, 3072]
        per_j = []
        for j in range(4):
            r = rows[:, 256 * j : 256 * j + 256]
            z = rows[:, 1024 + 256 * j : 1024 + 256 * j + 256]
            nn = rows[:, 2048 + 256 * j : 2048 + 256 * j + 256]
            per_j.append(np.concatenate([r, z, nn], axis=1))  # [128, 768]
        blocks.append(np.concatenate(per_j, axis=1))  # [128, 3072]
    WG = np.concatenate(blocks, axis=1).astype(ml_dtypes.bfloat16)  # [128,KG*3072]

    wfT = w_fc.T  # [1024, 768]
    WF = np.concatenate(
        [wfT[128 * k : 128 * k + 128] for k in range(KH)], axis=1
    ).astype(ml_dtypes.bfloat16)  # [128, KH*768]

    bfold = b_fc @ w_ih.T  # [3072]
    beff = b_ih + bfold
    brz_r = (beff + b_hh)[0:1024]
    brz_z = (beff + b_hh)[1024:2048]
    bin_v = beff[2048:3072]
    bhn_v = b_hh[2048:3072]

    def pack_bias(vec):  # [1024] -> [128, 256] rows 32j+b
        m = vec.reshape(4, 256)
        return np.repeat(m, 32, axis=0).astype(np.float32)  # [128, 256]

    BRZ = np.concatenate([pack_bias(brz_r), pack_bias(brz_z)], axis=1)
    BIN = pack_bias(bin_v)
    BHN = pack_bias(bhn_v)
    IDT = np.eye(128, dtype=np.float32)
    return WG, WF, BRZ, BIN, BHN, IDT


def _prep_core(x0, h0):
    """x0 [32,768] (already minus b_fc), h0 [32,1024] -> packed inits."""
    xr = x0.reshape(BC, 6, 128)
    X0I = np.ascontiguousarray(
        xr[:, :4, :].transpose(2, 1, 0).reshape(128, 128)
    ).astype(ml_dtypes.bfloat16)
    X1I = np.ascontiguousarray(
        xr[:, 4:6, :].transpose(2, 1, 0).reshape(128, 64)
    ).astype(ml_dtypes.bfloat16)
    hr = h0.reshape(BC, 4, 256)
    HPI = np.ascontiguousarray(hr.transpose(1, 0, 2).reshape(128, 256)).astype(
        np.float32
    )
    return X0I, X1I, HPI


def _build_in_maps(inputs):
    src = np.asarray(inputs["src"], np.float32)
    hidden = np.asarray(inputs["hidden"], np.float32)
    w_ih = np.asarray(inputs["w_ih"], np.float32)
    w_hh = np.asarray(inputs["w_hh"], np.float32)
    b_ih = np.asarray(inputs["b_ih"], np.float32)
    b_hh = np.asarray(inputs["b_hh"], np.float32)
    w_fc = np.asarray(inputs["w_fc"], np.float32)
    b_fc = np.asarray(inputs["b_fc"], np.float32)

    WG, WF, BRZ, BIN, BHN, IDT = _prep_shared(w_ih, w_hh, b_ih, b_hh, w_fc, b_fc)

    x0_all = src[0] - b_fc[None, :]  # [256, 768]
    h0_all = hidden[0]  # [256, 1024]
    in_maps = []
    for c in range(NCORES):
        sl = slice(BC * c, BC * (c + 1))
        X0I, X1I, HPI = _prep_core(x0_all[sl], h0_all[sl])
        cbf = np.concatenate(
            [WG, WF, X0I.astype(ml_dtypes.bfloat16), X1I.astype(ml_dtypes.bfloat16)],
            axis=1,
        )
        cf32 = np.concatenate([BRZ, BIN, BHN, IDT, HPI], axis=1).astype(np.float32)
        in_maps.append(dict(CBF=cbf, CF32=cf32))
    return in_maps


def kernel(src, tgt, hidden, w_ih, w_hh, b_ih, b_hh, w_fc, b_fc, **_kw):
    global _COMPILED
    b_fc = np.asarray(b_fc, np.float32)

    if _COMPILED is None:
        _COMPILED = _build_nc()
    nc = _COMPILED

    in_maps = _build_in_maps(
        dict(src=src, hidden=hidden, w_ih=w_ih, w_hh=w_hh, b_ih=b_ih,
             b_hh=b_hh, w_fc=w_fc, b_fc=b_fc)
    )

    res = run_bass_kernel_spmd(nc, in_maps, list(range(NCORES)))

    out = np.empty((T, B, O), np.float32)
    for c in range(NCORES):
        sl = slice(BC * c, BC * (c + 1))
        o0 = np.asarray(res.results[c]["O0"])  # [T, 128, 128]
        o1 = np.asarray(res.results[c]["O1"])  # [T, 64, 128]
        out[:, sl, 0:512] = (
            o0.reshape(T, 4, BC, 128).transpose(0, 2, 1, 3).reshape(T, BC, 512)
        )
        out[:, sl, 512:768] = (
            o1.reshape(T, 2, BC, 128).transpose(0, 2, 1, 3).reshape(T, BC, 256)
        )
    out += b_fc[None, None, :]
    return out



# revision 13
# speedup vs baseline: 1.3392x; 1.3392x over previous
"""GRU decoder Trainium2 kernel (data-parallel over batch, 8 cores).

Reference (per step t, PyTorch nn.GRU gate order r,z,n):
    gi = x @ w_ih.T + b_ih ; gh = h @ w_hh.T + b_hh
    r = sig(i_r + h_r); z = sig(i_z + h_z); n = tanh(i_n + r * h_n)
    h' = (1-z)*n + z*h ; y = h' @ w_fc.T + b_fc ; x <- y
Shapes: H=1024, O=768, B=256, T=256.  Each core handles 32 batch rows.

Key restructuring vs the straightforward mapping:
  * Since x_t = y_{t-1} = h'_{t-1} @ w_fc.T + b_fc, the input-side GRU
    matmuls fold into the hidden-side ones:
        r/z gates:  h' @ (w_fc.T @ w_ih_g.T + w_hh_g.T) + b_eff_g
        i_n:        h' @ (w_fc.T @ w_ih_n.T) + b_eff_in
        h_n:        h' @ w_hh_n.T + b_hh_n
    so every recurrent matmul contracts over H=1024 (not O+H=1792), and
    y_t itself is only needed for the OUTPUT -> off the critical path.
  * Step 0 gates are computed on the host from (src[0], hidden[0]).
  * Per core, batch=32 lives in the PE stationary free dim; 4 column
    groups (tile_position=(0,32j)) cover feature quadrants concurrently.
  * Gate regions are issued region-major in order r, h_n, i_n, z so the
    sigmoid/tanh chain for region g overlaps the matmuls of regions > g.
  * Per-step gate biases are seeded into PSUM with K=1 matmuls
    (ones[1,32] x bias_row[1,256]) as the start=True instruction of each
    accumulation group - no bias adds on the vector chain.
  * b_fc is added on the host at the end.
"""

import numpy as np
import ml_dtypes

import concourse.bass as bass
import concourse.bacc as bacc
import concourse.tile as tile
from concourse import mybir
from concourse.bass_utils import run_bass_kernel_spmd

H = 1024
O = 768
B = 256
T = 256
NCORES = 8
BC = B // NCORES  # 32 batch rows per core

KH = H // 128  # 8 contraction chunks
NGATE = 4      # regions r, hn, in, z (issue order)
YW = O // 4    # 192 y cols per quadrant

F32 = mybir.dt.float32
BF16 = mybir.dt.bfloat16
AF = mybir.ActivationFunctionType
ALU = mybir.AluOpType

_COMPILED = None

# bf16 const layout: WG | WF | ONES | BIAS
WG_N = NGATE * KH * 4 * 256   # 32768
WF_N = KH * 4 * YW            # 6144
NB = WG_N + WF_N + 32 + 4096  # 43040
# f32 const layout: G0 | HP0 | IDT | BV (packed per-quadrant bias tiles)
NF = NGATE * 256 + 256 + 128 + NGATE * 256  # 2432

# 'matmul': seed biases into PSUM with K=1 ones-row matmuls
# 'vector': add biases on the vector engine during the chain
BIAS_MODE = "vector"


def _hslice(hsb, k):
    """lhsT chunk k (h features 128k..128k+128) from packed h'^T tile."""
    c = 128 * (k % 2) + 32 * (k // 2)
    return hsb[:, c : c + 32]


def _build_nc():
    nc = bacc.Bacc("TRN2", target_bir_lowering=False, debug=False, num_devices=NCORES)

    cb = nc.declare_dram_parameter("CB", [128, NB], BF16, isOutput=False)
    cf = nc.declare_dram_parameter("CF", [128, NF], F32, isOutput=False)
    o = nc.declare_dram_parameter("O", [T, 128, YW], F32, isOutput=True)

    with tile.TileContext(nc) as tc:
        with (
            tc.tile_pool(name="wpool", bufs=1) as wpool,
            tc.tile_pool(name="state", bufs=2) as spool,
            tc.tile_pool(name="act", bufs=2) as apool,
            tc.tile_pool(name="gps", bufs=2, space="PSUM") as gpool,
            tc.tile_pool(name="tps", bufs=2, space="PSUM") as tpool,
            tc.tile_pool(name="yps", bufs=2, space="PSUM") as ypool,
        ):
            CB = wpool.tile([128, NB], BF16, tag="CB")
            CF = wpool.tile([128, NF], F32, tag="CF")
            nc.sync.dma_start(CB[:], cb[:])
            nc.sync.dma_start(CF[:], cf[:])
            WG = CB[:, 0:WG_N]
            WF = CB[:, WG_N : WG_N + WF_N]
            ONES = CB[0:1, WG_N + WF_N : WG_N + WF_N + 32]
            BIAS = CB[0:1, WG_N + WF_N + 32 : NB]
            G0 = CF[:, 0 : NGATE * 256]
            HPc = CF[:, NGATE * 256 : NGATE * 256 + 256]
            IDT = CF[:, NGATE * 256 + 256 : NGATE * 256 + 384]
            BV = CF[:, NGATE * 256 + 384 : NF]  # r|hn|in|z packed [128,256] each

            Hp = spool.tile([128, 256], F32, tag="Hp")
            nc.vector.tensor_copy(Hp[:], HPc)

            def chain(r_src, hn_src, in_src, z_src, hp, add_bias):
                """gates -> h' (scalar queue: rs, n, zs)"""
                if add_bias:
                    rb = apool.tile([128, 256], F32, tag="rb")
                    nc.vector.tensor_tensor(rb[:], r_src, BV[:, 0:256], ALU.add)
                    r_src = rb[:]
                rs = apool.tile([128, 256], F32, tag="rs")
                nc.scalar.activation(rs[:], r_src, AF.Sigmoid)
                if add_bias:
                    hb = apool.tile([128, 256], F32, tag="hb")
                    nc.vector.tensor_tensor(hb[:], hn_src, BV[:, 256:512], ALU.add)
                    hn_src = hb[:]
                rt = apool.tile([128, 256], F32, tag="rt")
                nc.vector.tensor_tensor(rt[:], rs[:], hn_src, ALU.mult)
                ns = apool.tile([128, 256], F32, tag="ns")
                nc.vector.tensor_tensor(ns[:], rt[:], in_src, ALU.add)
                if add_bias:
                    nb = apool.tile([128, 256], F32, tag="nb")
                    nc.vector.tensor_tensor(nb[:], ns[:], BV[:, 512:768], ALU.add)
                    ns = nb
                n = apool.tile([128, 256], F32, tag="n")
                nc.scalar.activation(n[:], ns[:], AF.Tanh)
                d = apool.tile([128, 256], F32, tag="d")
                nc.vector.tensor_tensor(d[:], hp[:], n[:], ALU.subtract)
                if add_bias:
                    zb = apool.tile([128, 256], F32, tag="zb")
                    nc.vector.tensor_tensor(zb[:], z_src, BV[:, 768:1024], ALU.add)
                    z_src = zb[:]
                zs = apool.tile([128, 256], F32, tag="zs")
                nc.scalar.activation(zs[:], z_src, AF.Sigmoid)
                e = apool.tile([128, 256], F32, tag="e")
                nc.vector.tensor_tensor(e[:], zs[:], d[:], ALU.mult)
                hp2 = spool.tile([128, 256], F32, tag="Hp")
                nc.vector.tensor_tensor(hp2[:], n[:], e[:], ALU.add)
                return hp2

            # step 0: gates computed host-side (biases already included)
            Hp = chain(
                G0[:, 0:256], G0[:, 256:512], G0[:, 512:768], G0[:, 768:1024],
                Hp, add_bias=False,
            )

            for t in range(T):
                # ---- h'_t^T for this step's y and next step's gates ----
                tp = tpool.tile([128, 256], F32, tag="tp")
                nc.tensor.transpose(tp[:, 0:128], Hp[:, 0:128], IDT)
                nc.tensor.transpose(tp[:, 128:256], Hp[:, 128:256], IDT)
                hsb = spool.tile([128, 256], BF16, tag="hsb")
                nc.scalar.activation(hsb[:], tp[:], AF.Copy)

                last = t == T - 1
                if not last:
                    # PSUM is bank-granular per tile: pack regions in
                    # [128,512] pairs (exactly one 2KB bank each).
                    g01 = gpool.tile([128, 512], F32, tag="g01")  # r | hn
                    g23 = gpool.tile([128, 512], F32, tag="g23")  # in | z
                    gsl = [
                        g01[:, 0:256], g01[:, 256:512],
                        g23[:, 0:256], g23[:, 256:512],
                    ]
                    if BIAS_MODE == "matmul":
                        # bias seed (start=True) for every region/quadrant
                        for g in range(NGATE):
                            for j in range(4):
                                nc.tensor.matmul(
                                    gsl[g][32 * j : 32 * j + 32, :],
                                    ONES[:, 0:32],
                                    BIAS[:, 1024 * g + 256 * j : 1024 * g + 256 * j + 256],
                                    start=True,
                                    stop=False,
                                    tile_position=(0, 32 * j),
                                )
                    # region-major gate matmuls: r, hn, in, z
                    for g in range(NGATE):
                        for k in range(KH):
                            lhsT = _hslice(hsb, k)
                            for j in range(4):
                                wofs = ((g * KH + k) * 4 + j) * 256
                                nc.tensor.matmul(
                                    gsl[g][32 * j : 32 * j + 32, :],
                                    lhsT,
                                    WG[:, wofs : wofs + 256],
                                    start=(BIAS_MODE == "vector" and k == 0),
                                    stop=(k == KH - 1),
                                    tile_position=(0, 32 * j),
                                )

                # ---- y_t = h'_t @ w_fc.T (output only; off critical path) ----
                yp = ypool.tile([128, YW], F32, tag="yp")
                for k in range(KH):
                    lhsT = _hslice(hsb, k)
                    for j in range(4):
                        wofs = (k * 4 + j) * YW
                        nc.tensor.matmul(
                            yp[32 * j : 32 * j + 32, :],
                            lhsT,
                            WF[:, wofs : wofs + YW],
                            start=(k == 0),
                            stop=(k == KH - 1),
                            tile_position=(0, 32 * j),
                        )

                if not last:
                    Hp = chain(
                        gsl[0], gsl[1], gsl[2], gsl[3], Hp,
                        add_bias=(BIAS_MODE == "vector"),
                    )

                ys = apool.tile([128, YW], F32, tag="ys")
                nc.vector.tensor_copy(ys[:], yp[:])
                nc.sync.dma_start(o[t], ys[:])

    nc.compile()
    return nc


def _pack_bat(M):
    """[32, 4*W] -> [128, W]: row 32j+b holds M[b, W*j : W*j+W]."""
    w = M.shape[1] // 4
    return np.ascontiguousarray(
        M.reshape(BC, 4, w).transpose(1, 0, 2).reshape(128, w)
    )


def _prep_shared(w_ih, w_hh, b_ih, b_hh, w_fc, b_fc):
    wihT = w_ih.T.astype(np.float64)  # [768, 3072]
    whhT = w_hh.T.astype(np.float64)  # [1024, 3072]
    wfcT = w_fc.T.astype(np.float64)  # [1024, 768]
    fold = wfcT @ wihT                # [1024, 3072]
    Wr = fold[:, 0:H] + whhT[:, 0:H]
    Wz = fold[:, H : 2 * H] + whhT[:, H : 2 * H]
    Win = fold[:, 2 * H : 3 * H]
    Whn = whhT[:, 2 * H : 3 * H]

    bfold = b_fc.astype(np.float64) @ wihT  # [3072]
    br = bfold[0:H] + b_ih[0:H] + b_hh[0:H]
    bz = bfold[H : 2 * H] + b_ih[H : 2 * H] + b_hh[H : 2 * H]
    bin_ = bfold[2 * H :] + b_ih[2 * H :]
    bhn = b_hh[2 * H :].astype(np.float64)

    blocks = []
    for G in (Wr, Whn, Win, Wz):  # region order r, hn, in, z
        for k in range(KH):
            for j in range(4):
                blocks.append(G[128 * k : 128 * k + 128, 256 * j : 256 * j + 256])
    WGp = np.concatenate(blocks, axis=1).astype(ml_dtypes.bfloat16)  # [128, 32768]

    yblocks = []
    for k in range(KH):
        for j in range(4):
            yblocks.append(wfcT[128 * k : 128 * k + 128, YW * j : YW * j + YW])
    WFp = np.concatenate(yblocks, axis=1).astype(ml_dtypes.bfloat16)  # [128, 6144]

    ones_col = np.zeros((128, 32), ml_dtypes.bfloat16)
    ones_col[0, :] = 1
    bias_col = np.zeros((128, 4096), ml_dtypes.bfloat16)
    bias_col[0, :] = np.concatenate([br, bhn, bin_, bz]).astype(ml_dtypes.bfloat16)

    CBp = np.concatenate([WGp, WFp, ones_col, bias_col], axis=1)  # [128, NB]
    assert CBp.shape[1] == NB
    IDT = np.eye(128, dtype=np.float32)

    def pack_bias(vec):  # [1024] -> [128, 256]: row 32j+b holds vec[256j:256j+256]
        return np.repeat(vec.reshape(4, 256), BC, axis=0).astype(np.float32)

    BVp = np.concatenate(
        [pack_bias(v) for v in (br, bhn, bin_, bz)], axis=1
    )  # [128, 1024] f32
    return CBp, IDT, BVp


def _build_in_maps(inputs):
    src = np.asarray(inputs["src"], np.float32)
    hidden = np.asarray(inputs["hidden"], np.float32)
    w_ih = np.asarray(inputs["w_ih"], np.float32)
    w_hh = np.asarray(inputs["w_hh"], np.float32)
    b_ih = np.asarray(inputs["b_ih"], np.float32)
    b_hh = np.asarray(inputs["b_hh"], np.float32)
    w_fc = np.asarray(inputs["w_fc"], np.float32)
    b_fc = np.asarray(inputs["b_fc"], np.float32)

    CBp, IDT, BVp = _prep_shared(w_ih, w_hh, b_ih, b_hh, w_fc, b_fc)

    # step-0 gates on host (f64): from x0=src[0], h0=hidden[0]
    x0 = src[0].astype(np.float64)   # [256, 768]
    h0 = hidden[0].astype(np.float64)  # [256, 1024]
    gi0 = x0 @ w_ih.T.astype(np.float64) + b_ih.astype(np.float64)
    gh0 = h0 @ w_hh.T.astype(np.float64) + b_hh.astype(np.float64)
    g0r = gi0[:, 0:H] + gh0[:, 0:H]
    g0z = gi0[:, H : 2 * H] + gh0[:, H : 2 * H]
    g0in = gi0[:, 2 * H :]
    g0hn = gh0[:, 2 * H :]

    in_maps = []
    for c in range(NCORES):
        sl = slice(BC * c, BC * (c + 1))
        G0 = np.concatenate(
            [
                _pack_bat(g0r[sl]),
                _pack_bat(g0hn[sl]),
                _pack_bat(g0in[sl]),
                _pack_bat(g0z[sl]),
            ],
            axis=1,
        )  # [128, 1024]
        HP0 = _pack_bat(h0[sl])  # [128, 256]
        CFp = np.concatenate([G0, HP0, IDT, BVp], axis=1).astype(np.float32)
        assert CFp.shape[1] == NF
        in_maps.append(dict(CB=CBp, CF=CFp))
    return in_maps


def kernel(src, tgt, hidden, w_ih, w_hh, b_ih, b_hh, w_fc, b_fc, **_kw):
    global _COMPILED
    b_fc = np.asarray(b_fc, np.float32)

    if _COMPILED is None:
        _COMPILED = _build_nc()
    nc = _COMPILED

    in_maps = _build_in_maps(
        dict(src=src, hidden=hidden, w_ih=w_ih, w_hh=w_hh, b_ih=b_ih,
             b_hh=b_hh, w_fc=w_fc, b_fc=b_fc)
    )

    res = run_bass_kernel_spmd(nc, in_maps, list(range(NCORES)))

    out = np.empty((T, B, O), np.float32)
    for c in range(NCORES):
        sl = slice(BC * c, BC * (c + 1))
        oc = np.asarray(res.results[c]["O"])  # [T, 128, 192]
        out[:, sl, :] = (
            oc.reshape(T, 4, BC, YW).transpose(0, 2, 1, 3).reshape(T, BC, O)
        )
    out += b_fc[None, None, :]
    return out


# revision 16
# speedup vs baseline: 1.7173x; 1.2824x over previous
"""GRU decoder Trainium2 kernel (data-parallel over batch, 8 cores).

Reference (per step t, PyTorch nn.GRU gate order r,z,n):
    gi = x @ w_ih.T + b_ih ; gh = h @ w_hh.T + b_hh
    r = sig(i_r + h_r); z = sig(i_z + h_z); n = tanh(i_n + r * h_n)
    h' = (1-z)*n + z*h ; y = h' @ w_fc.T + b_fc ; x <- y
Shapes: H=1024, O=768, B=256, T=256.  Each core handles 32 batch rows.

Key restructuring vs the straightforward mapping:
  * Since x_t = y_{t-1} = h'_{t-1} @ w_fc.T + b_fc, the input-side GRU
    matmuls fold into the hidden-side ones:
        r/z gates:  h' @ (w_fc.T @ w_ih_g.T + w_hh_g.T) + b_eff_g
        i_n:        h' @ (w_fc.T @ w_ih_n.T) + b_eff_in
        h_n:        h' @ w_hh_n.T + b_hh_n
    so every recurrent matmul contracts over H=1024 (not O+H=1792), and
    y_t itself is only needed for the OUTPUT -> off the critical path.
  * Step 0 gates are computed on the host from (src[0], hidden[0]).
  * Per core, batch=32 lives in the PE stationary free dim; 4 column
    groups (tile_position=(0,32j)) cover feature quadrants concurrently.
  * Gate regions are issued region-major in order r, h_n, i_n, z so the
    sigmoid/tanh chain for region g overlaps the matmuls of regions > g.
  * Per-step gate biases are seeded into PSUM with K=1 matmuls
    (ones[1,32] x bias_row[1,256]) as the start=True instruction of each
    accumulation group - no bias adds on the vector chain.
  * b_fc is added on the host at the end.
"""

import numpy as np
import ml_dtypes

import concourse.bass as bass
import concourse.bacc as bacc
import concourse.tile as tile
from concourse import mybir
from concourse.bass_utils import run_bass_kernel_spmd

H = 1024
O = 768
B = 256
T = 256
NCORES = 8
BC = B // NCORES  # 32 batch rows per core

KH = H // 128  # 8 contraction chunks
NGATE = 4      # regions r, hn, in, z (issue order)
YW = O // 4    # 192 y cols per quadrant

F32 = mybir.dt.float32
BF16 = mybir.dt.bfloat16
AF = mybir.ActivationFunctionType
ALU = mybir.AluOpType

_COMPILED = None

# bf16 const layout: WG | WF | ONES | BIAS
WG_N = NGATE * KH * 4 * 256   # 32768
WF_N = KH * 4 * YW            # 6144
NB = WG_N + WF_N + 32 + 4096  # 43040
# f32 const layout: G0 | HP0 | IDT | BV (packed per-quadrant bias tiles)
NF = NGATE * 256 + 256 + 128 + NGATE * 256  # 2432

# 'matmul': seed biases into PSUM with K=1 ones-row matmuls
# 'vector': add biases on the vector engine during the chain
BIAS_MODE = "matmul"


def _hslice(hsb, k):
    """lhsT chunk k (h features 128k..128k+128) from packed h'^T tile."""
    c = 128 * (k % 2) + 32 * (k // 2)
    return hsb[:, c : c + 32]


def _build_nc():
    nc = bacc.Bacc("TRN2", target_bir_lowering=False, debug=False, num_devices=NCORES)

    cb = nc.declare_dram_parameter("CB", [128, NB], BF16, isOutput=False)
    cf = nc.declare_dram_parameter("CF", [128, NF], F32, isOutput=False)
    o = nc.declare_dram_parameter("O", [T, 128, YW], F32, isOutput=True)

    with tile.TileContext(nc) as tc:
        with (
            tc.tile_pool(name="wpool", bufs=1) as wpool,
            tc.tile_pool(name="state", bufs=2) as spool,
            tc.tile_pool(name="act", bufs=2) as apool,
            tc.tile_pool(name="gps", bufs=2, space="PSUM") as gpool,
            tc.tile_pool(name="tps", bufs=2, space="PSUM") as tpool,
            tc.tile_pool(name="yps", bufs=2, space="PSUM") as ypool,
        ):
            CB = wpool.tile([128, NB], BF16, tag="CB")
            CF = wpool.tile([128, NF], F32, tag="CF")
            nc.sync.dma_start(CB[:], cb[:])
            nc.sync.dma_start(CF[:], cf[:])
            WG = CB[:, 0:WG_N]
            WF = CB[:, WG_N : WG_N + WF_N]
            ONES = CB[0:1, WG_N + WF_N : WG_N + WF_N + 32]
            BIAS = CB[0:1, WG_N + WF_N + 32 : NB]
            G0 = CF[:, 0 : NGATE * 256]
            HPc = CF[:, NGATE * 256 : NGATE * 256 + 256]
            IDT = CF[:, NGATE * 256 + 256 : NGATE * 256 + 384]
            BV = CF[:, NGATE * 256 + 384 : NF]  # r|hn|in|z packed [128,256] each

            Hp = spool.tile([128, 256], F32, tag="Hp")
            nc.vector.tensor_copy(Hp[:], HPc)

            def chain(r_src, hn_src, in_src, z_src, hp, add_bias):
                """gates -> h' (scalar queue: rs, n, zs)"""
                if add_bias:
                    rb = apool.tile([128, 256], F32, tag="rb")
                    nc.vector.tensor_tensor(rb[:], r_src, BV[:, 0:256], ALU.add)
                    r_src = rb[:]
                rs = apool.tile([128, 256], F32, tag="rs")
                nc.scalar.activation(rs[:], r_src, AF.Sigmoid)
                if add_bias:
                    hb = apool.tile([128, 256], F32, tag="hb")
                    nc.vector.tensor_tensor(hb[:], hn_src, BV[:, 256:512], ALU.add)
                    hn_src = hb[:]
                rt = apool.tile([128, 256], F32, tag="rt")
                nc.vector.tensor_tensor(rt[:], rs[:], hn_src, ALU.mult)
                ns = apool.tile([128, 256], F32, tag="ns")
                nc.vector.tensor_tensor(ns[:], rt[:], in_src, ALU.add)
                if add_bias:
                    nb = apool.tile([128, 256], F32, tag="nb")
                    nc.vector.tensor_tensor(nb[:], ns[:], BV[:, 512:768], ALU.add)
                    ns = nb
                n = apool.tile([128, 256], F32, tag="n")
                nc.scalar.activation(n[:], ns[:], AF.Tanh)
                d = apool.tile([128, 256], F32, tag="d")
                nc.vector.tensor_tensor(d[:], hp[:], n[:], ALU.subtract)
                if add_bias:
                    zb = apool.tile([128, 256], F32, tag="zb")
                    nc.vector.tensor_tensor(zb[:], z_src, BV[:, 768:1024], ALU.add)
                    z_src = zb[:]
                zs = apool.tile([128, 256], F32, tag="zs")
                nc.scalar.activation(zs[:], z_src, AF.Sigmoid)
                e = apool.tile([128, 256], F32, tag="e")
                nc.vector.tensor_tensor(e[:], zs[:], d[:], ALU.mult)
                hp2 = spool.tile([128, 256], F32, tag="Hp")
                nc.vector.tensor_tensor(hp2[:], n[:], e[:], ALU.add)
                return hp2

            # step 0: gates computed host-side (biases already included)
            Hp = chain(
                G0[:, 0:256], G0[:, 256:512], G0[:, 512:768], G0[:, 768:1024],
                Hp, add_bias=False,
            )

            for t in range(T):
                # ---- h'_t^T for this step's y and next step's gates ----
                tp = tpool.tile([128, 256], F32, tag="tp")
                nc.tensor.transpose(tp[:, 0:128], Hp[:, 0:128], IDT)
                nc.tensor.transpose(tp[:, 128:256], Hp[:, 128:256], IDT)
                hsb = spool.tile([128, 256], BF16, tag="hsb")
                nc.scalar.activation(hsb[:], tp[:], AF.Copy)

                last = t == T - 1
                if not last:
                    # PSUM is bank-granular per tile: pack regions in
                    # [128,512] pairs (exactly one 2KB bank each).
                    g01 = gpool.tile([128, 512], F32, tag="g01")  # r | hn
                    g23 = gpool.tile([128, 512], F32, tag="g23")  # in | z
                    gsl = [
                        g01[:, 0:256], g01[:, 256:512],
                        g23[:, 0:256], g23[:, 256:512],
                    ]
                    if BIAS_MODE == "matmul":
                        # ONE start=True bias MM per (bank, quadrant) seeding
                        # BOTH regions (N=512).  A second start in the same
                        # bank/partition-strip clears the whole strip's
                        # has_written bits and loses the earlier bias.
                        for bank, gp in enumerate((g01, g23)):
                            for j in range(4):
                                bofs = (bank * 4 + j) * 512
                                nc.tensor.matmul(
                                    gp[32 * j : 32 * j + 32, :],
                                    ONES[:, 0:32],
                                    BIAS[:, bofs : bofs + 512],
                                    start=True,
                                    stop=False,
                                    tile_position=(0, 32 * j),
                                )
                    # region-major gate matmuls: r, hn, in, z
                    for g in range(NGATE):
                        for k in range(KH):
                            lhsT = _hslice(hsb, k)
                            for j in range(4):
                                wofs = ((g * KH + k) * 4 + j) * 256
                                nc.tensor.matmul(
                                    gsl[g][32 * j : 32 * j + 32, :],
                                    lhsT,
                                    WG[:, wofs : wofs + 256],
                                    start=(BIAS_MODE == "vector" and k == 0),
                                    stop=(k == KH - 1),
                                    tile_position=(0, 32 * j),
                                )

                # ---- y_t = h'_t @ w_fc.T (output only; off critical path) ----
                yp = ypool.tile([128, YW], F32, tag="yp")
                for k in range(KH):
                    lhsT = _hslice(hsb, k)
                    for j in range(4):
                        wofs = (k * 4 + j) * YW
                        nc.tensor.matmul(
                            yp[32 * j : 32 * j + 32, :],
                            lhsT,
                            WF[:, wofs : wofs + YW],
                            start=(k == 0),
                            stop=(k == KH - 1),
                            tile_position=(0, 32 * j),
                        )

                if not last:
                    Hp = chain(
                        gsl[0], gsl[1], gsl[2], gsl[3], Hp,
                        add_bias=(BIAS_MODE == "vector"),
                    )

                ys = apool.tile([128, YW], F32, tag="ys")
                nc.vector.tensor_copy(ys[:], yp[:])
                nc.sync.dma_start(o[t], ys[:])

    nc.compile()
    return nc


def _pack_bat(M):
    """[32, 4*W] -> [128, W]: row 32j+b holds M[b, W*j : W*j+W]."""
    w = M.shape[1] // 4
    return np.ascontiguousarray(
        M.reshape(BC, 4, w).transpose(1, 0, 2).reshape(128, w)
    )


def _prep_shared(w_ih, w_hh, b_ih, b_hh, w_fc, b_fc):
    wihT = w_ih.T.astype(np.float64)  # [768, 3072]
    whhT = w_hh.T.astype(np.float64)  # [1024, 3072]
    wfcT = w_fc.T.astype(np.float64)  # [1024, 768]
    fold = wfcT @ wihT                # [1024, 3072]
    Wr = fold[:, 0:H] + whhT[:, 0:H]
    Wz = fold[:, H : 2 * H] + whhT[:, H : 2 * H]
    Win = fold[:, 2 * H : 3 * H]
    Whn = whhT[:, 2 * H : 3 * H]

    bfold = b_fc.astype(np.float64) @ wihT  # [3072]
    br = bfold[0:H] + b_ih[0:H] + b_hh[0:H]
    bz = bfold[H : 2 * H] + b_ih[H : 2 * H] + b_hh[H : 2 * H]
    bin_ = bfold[2 * H :] + b_ih[2 * H :]
    bhn = b_hh[2 * H :].astype(np.float64)

    blocks = []
    for G in (Wr, Whn, Win, Wz):  # region order r, hn, in, z
        for k in range(KH):
            for j in range(4):
                blocks.append(G[128 * k : 128 * k + 128, 256 * j : 256 * j + 256])
    WGp = np.concatenate(blocks, axis=1).astype(ml_dtypes.bfloat16)  # [128, 32768]

    yblocks = []
    for k in range(KH):
        for j in range(4):
            yblocks.append(wfcT[128 * k : 128 * k + 128, YW * j : YW * j + YW])
    WFp = np.concatenate(yblocks, axis=1).astype(ml_dtypes.bfloat16)  # [128, 6144]

    ones_col = np.zeros((128, 32), ml_dtypes.bfloat16)
    ones_col[0, :] = 1
    # bias layout: (bank*4+j)*512 -> [bankA[256j:+256] | bankB[256j:+256]]
    bias_row = np.empty(4096, np.float64)
    for bank, (bA, bB) in enumerate(((br, bhn), (bin_, bz))):
        for j in range(4):
            o = (bank * 4 + j) * 512
            bias_row[o : o + 256] = bA[256 * j : 256 * j + 256]
            bias_row[o + 256 : o + 512] = bB[256 * j : 256 * j + 256]
    bias_col = np.zeros((128, 4096), ml_dtypes.bfloat16)
    bias_col[0, :] = bias_row.astype(ml_dtypes.bfloat16)

    CBp = np.concatenate([WGp, WFp, ones_col, bias_col], axis=1)  # [128, NB]
    assert CBp.shape[1] == NB
    IDT = np.eye(128, dtype=np.float32)

    def pack_bias(vec):  # [1024] -> [128, 256]: row 32j+b holds vec[256j:256j+256]
        return np.repeat(vec.reshape(4, 256), BC, axis=0).astype(np.float32)

    BVp = np.concatenate(
        [pack_bias(v) for v in (br, bhn, bin_, bz)], axis=1
    )  # [128, 1024] f32
    return CBp, IDT, BVp


def _build_in_maps(inputs):
    src = np.asarray(inputs["src"], np.float32)
    hidden = np.asarray(inputs["hidden"], np.float32)
    w_ih = np.asarray(inputs["w_ih"], np.float32)
    w_hh = np.asarray(inputs["w_hh"], np.float32)
    b_ih = np.asarray(inputs["b_ih"], np.float32)
    b_hh = np.asarray(inputs["b_hh"], np.float32)
    w_fc = np.asarray(inputs["w_fc"], np.float32)
    b_fc = np.asarray(inputs["b_fc"], np.float32)

    CBp, IDT, BVp = _prep_shared(w_ih, w_hh, b_ih, b_hh, w_fc, b_fc)

    # step-0 gates on host (f64): from x0=src[0], h0=hidden[0]
    x0 = src[0].astype(np.float64)   # [256, 768]
    h0 = hidden[0].astype(np.float64)  # [256, 1024]
    gi0 = x0 @ w_ih.T.astype(np.float64) + b_ih.astype(np.float64)
    gh0 = h0 @ w_hh.T.astype(np.float64) + b_hh.astype(np.float64)
    g0r = gi0[:, 0:H] + gh0[:, 0:H]
    g0z = gi0[:, H : 2 * H] + gh0[:, H : 2 * H]
    g0in = gi0[:, 2 * H :]
    g0hn = gh0[:, 2 * H :]

    in_maps = []
    for c in range(NCORES):
        sl = slice(BC * c, BC * (c + 1))
        G0 = np.concatenate(
            [
                _pack_bat(g0r[sl]),
                _pack_bat(g0hn[sl]),
                _pack_bat(g0in[sl]),
                _pack_bat(g0z[sl]),
            ],
            axis=1,
        )  # [128, 1024]
        HP0 = _pack_bat(h0[sl])  # [128, 256]
        CFp = np.concatenate([G0, HP0, IDT, BVp], axis=1).astype(np.float32)
        assert CFp.shape[1] == NF
        in_maps.append(dict(CB=CBp, CF=CFp))
    return in_maps


def kernel(src, tgt, hidden, w_ih, w_hh, b_ih, b_hh, w_fc, b_fc, **_kw):
    global _COMPILED
    b_fc = np.asarray(b_fc, np.float32)

    if _COMPILED is None:
        _COMPILED = _build_nc()
    nc = _COMPILED

    in_maps = _build_in_maps(
        dict(src=src, hidden=hidden, w_ih=w_ih, w_hh=w_hh, b_ih=b_ih,
             b_hh=b_hh, w_fc=w_fc, b_fc=b_fc)
    )

    res = run_bass_kernel_spmd(nc, in_maps, list(range(NCORES)))

    out = np.empty((T, B, O), np.float32)
    for c in range(NCORES):
        sl = slice(BC * c, BC * (c + 1))
        oc = np.asarray(res.results[c]["O"])  # [T, 128, 192]
        out[:, sl, :] = (
            oc.reshape(T, 4, BC, YW).transpose(0, 2, 1, 3).reshape(T, BC, O)
        )
    out += b_fc[None, None, :]
    return out


# revision 20
# speedup vs baseline: 2.0642x; 1.2020x over previous
"""GRU decoder Trainium2 kernel (data-parallel over batch, 8 cores).

Reference (per step t, PyTorch nn.GRU gate order r,z,n):
    gi = x @ w_ih.T + b_ih ; gh = h @ w_hh.T + b_hh
    r = sig(i_r + h_r); z = sig(i_z + h_z); n = tanh(i_n + r * h_n)
    h' = (1-z)*n + z*h ; y = h' @ w_fc.T + b_fc ; x <- y
Shapes: H=1024, O=768, B=256, T=256.  Each core handles 32 batch rows.

Key restructuring vs the straightforward mapping:
  * Since x_t = y_{t-1} = h'_{t-1} @ w_fc.T + b_fc, the input-side GRU
    matmuls fold into the hidden-side ones:
        r/z gates:  h' @ (w_fc.T @ w_ih_g.T + w_hh_g.T) + b_eff_g
        i_n:        h' @ (w_fc.T @ w_ih_n.T) + b_eff_in
        h_n:        h' @ w_hh_n.T + b_hh_n
    so every recurrent matmul contracts over H=1024 (not O+H=1792), and
    y_t itself is only needed for the OUTPUT -> off the critical path.
  * Step 0 gates are computed on the host from (src[0], hidden[0]).
  * Per core, batch=32 lives in the PE stationary free dim; 4 column
    groups (tile_position=(0,32j)) cover feature quadrants concurrently.
  * Gate regions are issued region-major in order r, h_n, i_n, z so the
    sigmoid/tanh chain for region g overlaps the matmuls of regions > g.
  * Per-step gate biases are seeded into PSUM with K=1 matmuls
    (ones[1,32] x bias_row[1,256]) as the start=True instruction of each
    accumulation group - no bias adds on the vector chain.
  * b_fc is added on the host at the end.
"""

import numpy as np
import ml_dtypes

import concourse.bass as bass
import concourse.bacc as bacc
import concourse.tile as tile
from concourse import mybir
from concourse.bass_utils import run_bass_kernel_spmd

H = 1024
O = 768
B = 256
T = 256
NCORES = 8
BC = B // NCORES  # 32 batch rows per core

KH = H // 128  # 8 contraction chunks
NGATE = 4      # regions r, hn, in, z (issue order)
YW = O // 4    # 192 y cols per quadrant

F32 = mybir.dt.float32
BF16 = mybir.dt.bfloat16
AF = mybir.ActivationFunctionType
ALU = mybir.AluOpType

_COMPILED = None

# bf16 const layout: WG | WF | ONES | BIAS
WG_N = NGATE * KH * 4 * 256   # 32768
WF_N = KH * 4 * YW            # 6144
NB = WG_N + WF_N + 32 + 4096  # 43040
# f32 const layout: G0 | HP0 | IDT | BV (packed per-quadrant bias tiles)
NF = NGATE * 256 + 256 + 128 + NGATE * 256  # 2432

# 'matmul': seed biases into PSUM with K=1 ones-row matmuls
# 'vector': add biases on the vector engine during the chain
BIAS_MODE = "matmul"


def _hslice(hsb0, hsb1, k):
    """lhsT chunk k (h features 128k..128k+128) from packed h'^T half-tiles."""
    c = 32 * (k // 2)
    src = hsb0 if k % 2 == 0 else hsb1
    return src[:, c : c + 32]


def _build_nc():
    nc = bacc.Bacc("TRN2", target_bir_lowering=False, debug=False, num_devices=NCORES)

    cb = nc.declare_dram_parameter("CB", [128, NB], BF16, isOutput=False)
    cf = nc.declare_dram_parameter("CF", [128, NF], F32, isOutput=False)
    o = nc.declare_dram_parameter("O", [T, 128, YW], F32, isOutput=True)

    with tile.TileContext(nc) as tc:
        with (
            tc.tile_pool(name="wpool", bufs=1) as wpool,
            tc.tile_pool(name="state", bufs=2) as spool,
            tc.tile_pool(name="act", bufs=2) as apool,
            tc.tile_pool(name="gps", bufs=1, space="PSUM") as gpool,
            tc.tile_pool(name="tps", bufs=2, space="PSUM") as tpool,
            tc.tile_pool(name="yps", bufs=2, space="PSUM") as ypool,
        ):
            CB = wpool.tile([128, NB], BF16, tag="CB")
            CF = wpool.tile([128, NF], F32, tag="CF")
            nc.sync.dma_start(CB[:], cb[:])
            nc.sync.dma_start(CF[:], cf[:])
            WG = CB[:, 0:WG_N]
            WF = CB[:, WG_N : WG_N + WF_N]
            ONES = CB[0:1, WG_N + WF_N : WG_N + WF_N + 32]
            BIAS = CB[0:1, WG_N + WF_N + 32 : NB]
            G0 = CF[:, 0 : NGATE * 256]
            HPc = CF[:, NGATE * 256 : NGATE * 256 + 256]
            IDT = CF[:, NGATE * 256 + 256 : NGATE * 256 + 384]
            BV = CF[:, NGATE * 256 + 384 : NF]  # r|hn|in|z packed [128,256] each

            Hp = spool.tile([128, 256], F32, tag="Hp")
            nc.vector.tensor_copy(Hp[:], HPc)

            def chain(r_src, hn_src, in_src, z_src, hp, add_bias):
                """gates -> h' (scalar queue: rs, n, zs)"""
                if add_bias:
                    rb = apool.tile([128, 256], F32, tag="rb")
                    nc.vector.tensor_tensor(rb[:], r_src, BV[:, 0:256], ALU.add)
                    r_src = rb[:]
                rs = apool.tile([128, 256], F32, tag="rs")
                nc.scalar.activation(rs[:], r_src, AF.Sigmoid)
                if add_bias:
                    hb = apool.tile([128, 256], F32, tag="hb")
                    nc.vector.tensor_tensor(hb[:], hn_src, BV[:, 256:512], ALU.add)
                    hn_src = hb[:]
                rt = apool.tile([128, 256], F32, tag="rt")
                nc.vector.tensor_tensor(rt[:], rs[:], hn_src, ALU.mult)
                ns = apool.tile([128, 256], F32, tag="ns")
                nc.vector.tensor_tensor(ns[:], rt[:], in_src, ALU.add)
                if add_bias:
                    nb = apool.tile([128, 256], F32, tag="nb")
                    nc.vector.tensor_tensor(nb[:], ns[:], BV[:, 512:768], ALU.add)
                    ns = nb
                n = apool.tile([128, 256], F32, tag="n")
                nc.scalar.activation(n[:], ns[:], AF.Tanh)
                d = apool.tile([128, 256], F32, tag="d")
                nc.vector.tensor_tensor(d[:], hp[:], n[:], ALU.subtract)
                if add_bias:
                    zb = apool.tile([128, 256], F32, tag="zb")
                    nc.vector.tensor_tensor(zb[:], z_src, BV[:, 768:1024], ALU.add)
                    z_src = zb[:]
                zs = apool.tile([128, 256], F32, tag="zs")
                nc.scalar.activation(zs[:], z_src, AF.Sigmoid)
                e = apool.tile([128, 256], F32, tag="e")
                nc.vector.tensor_tensor(e[:], zs[:], d[:], ALU.mult)
                hp2 = spool.tile([128, 256], F32, tag="Hp")
                nc.vector.tensor_tensor(hp2[:], n[:], e[:], ALU.add)
                return hp2

            # step 0: gates computed host-side (biases already included)
            Hp = chain(
                G0[:, 0:256], G0[:, 256:512], G0[:, 512:768], G0[:, 768:1024],
                Hp, add_bias=False,
            )

            for t in range(T):
                # ---- h'_t^T for this step's y and next step's gates ----
                # Two half-tiles: deps are tile-granular, so separate tiles
                # let even-k matmuls start after the first transpose+cast.
                tp = tpool.tile([128, 256], F32, tag="tp")
                nc.tensor.transpose(tp[:, 0:128], Hp[:, 0:128], IDT)
                nc.tensor.transpose(tp[:, 128:256], Hp[:, 128:256], IDT)
                hsb0 = spool.tile([128, 128], BF16, tag="hsb0")
                hsb1 = spool.tile([128, 128], BF16, tag="hsb1")
                nc.scalar.activation(hsb0[:], tp[:, 0:128], AF.Copy)
                nc.vector.tensor_copy(hsb1[:], tp[:, 128:256])

                last = t == T - 1
                if not last:
                    # One PSUM bank per region (bufs=1: the chain drains
                    # within the step, so no double buffering needed) ->
                    # per-region dep granularity for early chain start.
                    gsl = [
                        gpool.tile([128, 256], F32, tag=f"g{g}", name=f"g{g}")
                        for g in range(NGATE)
                    ]
                    if BIAS_MODE == "matmul":
                        # bias seed (start=True); each region has its own
                        # bank so starts never clear another region's bias
                        for g in range(NGATE):
                            for j in range(4):
                                bofs = 1024 * g + 256 * j
                                nc.tensor.matmul(
                                    gsl[g][32 * j : 32 * j + 32, :],
                                    ONES[:, 0:32],
                                    BIAS[:, bofs : bofs + 256],
                                    start=True,
                                    stop=False,
                                    tile_position=(0, 32 * j),
                                )
                    # region-major gate matmuls: r, hn, in, z
                    for g in range(NGATE):
                        for k in range(KH):
                            lhsT = _hslice(hsb0, hsb1, k)
                            for j in range(4):
                                wofs = ((g * KH + k) * 4 + j) * 256
                                nc.tensor.matmul(
                                    gsl[g][32 * j : 32 * j + 32, :],
                                    lhsT,
                                    WG[:, wofs : wofs + 256],
                                    start=(BIAS_MODE == "vector" and k == 0),
                                    stop=(k == KH - 1),
                                    tile_position=(0, 32 * j),
                                )
                    gsl = [g[:] for g in gsl]

                # ---- y_t = h'_t @ w_fc.T (output only; off critical path) ----
                yp = ypool.tile([128, YW], F32, tag="yp")
                for k in range(KH):
                    lhsT = _hslice(hsb0, hsb1, k)
                    for j in range(4):
                        wofs = (k * 4 + j) * YW
                        nc.tensor.matmul(
                            yp[32 * j : 32 * j + 32, :],
                            lhsT,
                            WF[:, wofs : wofs + YW],
                            start=(k == 0),
                            stop=(k == KH - 1),
                            tile_position=(0, 32 * j),
                        )

                if not last:
                    Hp = chain(
                        gsl[0], gsl[1], gsl[2], gsl[3], Hp,
                        add_bias=(BIAS_MODE == "vector"),
                    )

                ys = apool.tile([128, YW], F32, tag="ys")
                nc.vector.tensor_copy(ys[:], yp[:])
                nc.sync.dma_start(o[t], ys[:])

    nc.compile()
    return nc


def _pack_bat(M):
    """[32, 4*W] -> [128, W]: row 32j+b holds M[b, W*j : W*j+W]."""
    w = M.shape[1] // 4
    return np.ascontiguousarray(
        M.reshape(BC, 4, w).transpose(1, 0, 2).reshape(128, w)
    )


def _prep_shared(w_ih, w_hh, b_ih, b_hh, w_fc, b_fc):
    wihT = w_ih.T.astype(np.float64)  # [768, 3072]
    whhT = w_hh.T.astype(np.float64)  # [1024, 3072]
    wfcT = w_fc.T.astype(np.float64)  # [1024, 768]
    fold = wfcT @ wihT                # [1024, 3072]
    Wr = fold[:, 0:H] + whhT[:, 0:H]
    Wz = fold[:, H : 2 * H] + whhT[:, H : 2 * H]
    Win = fold[:, 2 * H : 3 * H]
    Whn = whhT[:, 2 * H : 3 * H]

    bfold = b_fc.astype(np.float64) @ wihT  # [3072]
    br = bfold[0:H] + b_ih[0:H] + b_hh[0:H]
    bz = bfold[H : 2 * H] + b_ih[H : 2 * H] + b_hh[H : 2 * H]
    bin_ = bfold[2 * H :] + b_ih[2 * H :]
    bhn = b_hh[2 * H :].astype(np.float64)

    blocks = []
    for G in (Wr, Whn, Win, Wz):  # region order r, hn, in, z
        for k in range(KH):
            for j in range(4):
                blocks.append(G[128 * k : 128 * k + 128, 256 * j : 256 * j + 256])
    WGp = np.concatenate(blocks, axis=1).astype(ml_dtypes.bfloat16)  # [128, 32768]

    yblocks = []
    for k in range(KH):
        for j in range(4):
            yblocks.append(wfcT[128 * k : 128 * k + 128, YW * j : YW * j + YW])
    WFp = np.concatenate(yblocks, axis=1).astype(ml_dtypes.bfloat16)  # [128, 6144]

    ones_col = np.zeros((128, 32), ml_dtypes.bfloat16)
    ones_col[0, :] = 1
    # bias layout: 1024*g + 256*j (regions r, hn, in, z)
    bias_col = np.zeros((128, 4096), ml_dtypes.bfloat16)
    bias_col[0, :] = np.concatenate([br, bhn, bin_, bz]).astype(ml_dtypes.bfloat16)

    CBp = np.concatenate([WGp, WFp, ones_col, bias_col], axis=1)  # [128, NB]
    assert CBp.shape[1] == NB
    IDT = np.eye(128, dtype=np.float32)

    def pack_bias(vec):  # [1024] -> [128, 256]: row 32j+b holds vec[256j:256j+256]
        return np.repeat(vec.reshape(4, 256), BC, axis=0).astype(np.float32)

    BVp = np.concatenate(
        [pack_bias(v) for v in (br, bhn, bin_, bz)], axis=1
    )  # [128, 1024] f32
    return CBp, IDT, BVp


def _build_in_maps(inputs):
    src = np.asarray(inputs["src"], np.float32)
    hidden = np.asarray(inputs["hidden"], np.float32)
    w_ih = np.asarray(inputs["w_ih"], np.float32)
    w_hh = np.asarray(inputs["w_hh"], np.float32)
    b_ih = np.asarray(inputs["b_ih"], np.float32)
    b_hh = np.asarray(inputs["b_hh"], np.float32)
    w_fc = np.asarray(inputs["w_fc"], np.float32)
    b_fc = np.asarray(inputs["b_fc"], np.float32)

    CBp, IDT, BVp = _prep_shared(w_ih, w_hh, b_ih, b_hh, w_fc, b_fc)

    # step-0 gates on host (f64): from x0=src[0], h0=hidden[0]
    x0 = src[0].astype(np.float64)   # [256, 768]
    h0 = hidden[0].astype(np.float64)  # [256, 1024]
    gi0 = x0 @ w_ih.T.astype(np.float64) + b_ih.astype(np.float64)
    gh0 = h0 @ w_hh.T.astype(np.float64) + b_hh.astype(np.float64)
    g0r = gi0[:, 0:H] + gh0[:, 0:H]
    g0z = gi0[:, H : 2 * H] + gh0[:, H : 2 * H]
    g0in = gi0[:, 2 * H :]
    g0hn = gh0[:, 2 * H :]

    in_maps = []
    for c in range(NCORES):
        sl = slice(BC * c, BC * (c + 1))
        G0 = np.concatenate(
            [
                _pack_bat(g0r[sl]),
                _pack_bat(g0hn[sl]),
                _pack_bat(g0in[sl]),
                _pack_bat(g0z[sl]),
            ],
            axis=1,
        )  # [128, 1024]
        HP0 = _pack_bat(h0[sl])  # [128, 256]
        CFp = np.concatenate([G0, HP0, IDT, BVp], axis=1).astype(np.float32)
        assert CFp.shape[1] == NF
        in_maps.append(dict(CB=CBp, CF=CFp))
    return in_maps


def kernel(src, tgt, hidden, w_ih, w_hh, b_ih, b_hh, w_fc, b_fc, **_kw):
    global _COMPILED
    b_fc = np.asarray(b_fc, np.float32)

    if _COMPILED is None:
        _COMPILED = _build_nc()
    nc = _COMPILED

    in_maps = _build_in_maps(
        dict(src=src, hidden=hidden, w_ih=w_ih, w_hh=w_hh, b_ih=b_ih,
             b_hh=b_hh, w_fc=w_fc, b_fc=b_fc)
    )

    res = run_bass_kernel_spmd(nc, in_maps, list(range(NCORES)))

    out = np.empty((T, B, O), np.float32)
    for c in range(NCORES):
        sl = slice(BC * c, BC * (c + 1))
        oc = np.asarray(res.results[c]["O"])  # [T, 128, 192]
        out[:, sl, :] = (
            oc.reshape(T, 4, BC, YW).transpose(0, 2, 1, 3).reshape(T, BC, O)
        )
    out += b_fc[None, None, :]
    return out
